# revision 15
# baseline (speedup 1.0000x reference)
"""Trainium2 Bass kernel for the Chambolle-Pock-style primal/dual stencil loop.

Math (per image, H=W=1024, EPS=0.5, TAU=0.5, 10 iterations):
    u = sigmoid(o/EPS); q = 0
    repeat 10x:
        q  = relu(q - TAU*(vf1*Dy(u) + vf0*Dx(u)))   # forward diffs, zero pad
        Tq = BDy(vf1*q) + BDx(vf0*q)                  # backward diffs, zero pad
        u  = sigmoid((o - Tq)/EPS)
    return (o - Tq)/EPS

Rescaling: with qh = 2*sqrt(2)*q, g = vf/sqrt(2), o2 = 2*o, s = 2(o - Tq),
t = tanh(s/2)  (u = 0.5 + 0.5*t; zero-padding of u becomes (-1)-padding of t):
    qh = relu(qh - g1*(St - t) - g0*(Rt - t))        # S: y+1 shift, R: x+1
    s  = o2 - (a - Sa) - (b - Rb),  a = g1*qh, b = g0*qh   # backward diffs
    t  = tanh(s/2)
and the final output is s.

Three-engine split (all state fp16; validated rel-L2 vs the fp32 jax
reference ~9e-3, under the 2e-2 gate — the error is early relu
decision-boundary noise, not accumulation):
  - 7 tensor-tensor ops/iter run column-split on DVE (cols 0:838, fp16
    2x_1p mode, 0.52 ns/elem) + GpSimd/Pool (cols 838:1024, TT at 0.42 of
    0.83 ns/elem); the split latency-balances the two engines per op:
      d1 = St - t;  t1 = ng1*d1;  d2 = Rt - t;  t2 = ng0*d2
      na = ng1*qh;  nb = ng0*qh;  db = nb - Rnb          (ng = -g, host-side)
  - the remaining 7 adds/iter run on the otherwise-idle TensorEngine as
    +/-identity matmuls accumulating in PSUM (fp32 accumulation - better
    numerics than fp16 adds), one plane ([128,1024] fp32 = 2 PSUM banks)
    per chunk, double-buffered dual + primal pools = all 8 banks:
      psum1 = I*qh + I*t1 + I*t2                 -> ScalarE relu -> qh (fp16)
      psum2 = I*o2 + I*na - I*Sna + I*db         -> ScalarE tanh(x/2) -> t
    (Sna is the plane-shifted read of na, so the "da" diff costs a PE pass
    instead of a DVE op; db keeps the DVE form except plane 7, which goes
    via an extra PE pass pair - that balance measured best.)
  - last iteration: psum2 chunks are Copy-drained to fp16 and DMA'd out on
    the SP queue (a scalar-queue DMA issue would block ActE's sequencer
    between Copy drains); the host upcasts to fp32.

Boundary handling: image row y = 8*p + i -> partition p (0..127), plane i
(0..7) in the free dim.  Row shifts are free-dim plane offsets; only the
plane7 -> next-partition boundary crosses partitions, via a tiny SBUF->SBUF
DMA per iteration (t: plane 8 of sut; -a: plane 0 of nat, with a dedicated
plane-7 row multiply emitted first so the DMA fires early).  Column shifts
use guard columns (sut col W = -1; nbt col 0 = 0).  Only guard regions are
memset; interiors are fully overwritten (and the iteration-0 dual skips the
I*qh pass since qh0 = 0, so qht needs no init at all).

Sharding: pure data parallel, one image per NeuronCore (B=8 over 8 cores),
ng0/ng1 broadcast.
"""

import numpy as np

import concourse.bacc as bacc
import concourse.mybir as mybir
from concourse.tile import TileContext
from concourse import bass_utils

F32 = mybir.dt.float32
F16 = mybir.dt.float16
AF = mybir.ActivationFunctionType

B, H, W = 8, 1024, 1024
P = 128          # SBUF partitions
NP = H // P      # planes per partition = 8
WG = W + 2       # plane width incl. one guard column (+1 pad to even)
XS = 838         # DVE handles cols [0, XS), Pool cols [XS, W)
NB = 2           # planes per tensor-tensor block
MAXITER = 10

_CACHE = {}
LAST_RESULTS = None  # BassKernelResults of the most recent run (for test.py)


def _build(reps=1):
    nc = bacc.Bacc("TRN2", target_bir_lowering=False, debug=False)

    o2_d = nc.dram_tensor("o2", [H, W], F16, kind="ExternalInput").ap()
    g0_d = nc.dram_tensor("ng0", [H, W], F16, kind="ExternalInput").ap()
    g1_d = nc.dram_tensor("ng1", [H, W], F16, kind="ExternalInput").ap()
    eye_d = nc.dram_tensor("eye", [P, P], F16, kind="ExternalInput").ap()
    neye_d = nc.dram_tensor("neye", [P, P], F16, kind="ExternalInput").ap()
    out_d = nc.dram_tensor("out", [H, W], F16, kind="ExternalOutput").ap()

    # (H, W) -> (p, i, x) with y = 8*p + i
    o2_v = o2_d.rearrange("(p i) x -> p i x", i=NP)
    g0_v = g0_d.rearrange("(p i) x -> p i x", i=NP)
    g1_v = g1_d.rearrange("(p i) x -> p i x", i=NP)
    out_v = out_d.rearrange("(p i) x -> p i x", i=NP)

    v = nc.vector
    gp = nc.gpsimd
    act = nc.scalar
    pe = nc.tensor

    with TileContext(nc) as tc:
        with (
            tc.tile_pool(name="main", bufs=1) as pool,
            tc.tile_pool(name="ps1", bufs=2, space="PSUM") as psp1,
            tc.tile_pool(name="ps2", bufs=2, space="PSUM") as psp2,
        ):
            o2t = pool.tile([P, NP, W], F16)
            ng0t = pool.tile([P, NP, W], F16)
            ng1t = pool.tile([P, NP, W], F16)
            qht = pool.tile([P, NP, W], F16)
            eyet = pool.tile([P, P], F16)
            neyet = pool.tile([P, P], F16)
            # sut: planes 0..7 = t data (col W = -1 guard for x+1 reads),
            # plane 8 = boundary row t[8p+8, x] (partition 127 stays -1)
            sut = pool.tile([P, NP + 1, WG], F16)
            # d1/d2: diff scratch, overwritten in place by t1/t2
            d1t = pool.tile([P, NP, W], F16)
            d2t = pool.tile([P, NP, W], F16)
            # na: planes 1..8 = -a data, plane 0 = boundary row -a[8p-1, x]
            nat = pool.tile([P, NP + 1, W], F16)
            # nb: cols 1..W = -b data, col 0 = zero guard for x-1 reads
            nbt = pool.tile([P, NP, WG], F16)
            dbt = pool.tile([P, NP, W], F16)
            outst = pool.tile([P, 4, W], F16)   # last-iter drain staging

            ENG = [(v, 0, XS), (gp, XS, W)]
            BLOCKS = [(b * NB, (b + 1) * NB) for b in range(NP // NB)]

            def u_(lo, hi, c0, c1):
                return sut[:, lo:hi, c0:c1]

            def unr(lo, hi, c0, c1):   # t[y+1, x] (plane 8 = boundary)
                return sut[:, lo + 1 : hi + 1, c0:c1]

            def unc(lo, hi, c0, c1):   # t[y, x+1] (col W = -1 guard)
                return sut[:, lo:hi, c0 + 1 : c1 + 1]

            def mk(tile):
                def f(lo, hi, c0, c1):
                    return tile[:, lo:hi, c0:c1]
                return f

            o2_, ng0_, ng1_, qh_ = mk(o2t), mk(ng0t), mk(ng1t), mk(qht)
            d1_, d2_, db_ = mk(d1t), mk(d2t), mk(dbt)

            def na_(lo, hi, c0, c1):     # -a data (planes 1..8)
                return nat[:, lo + 1 : hi + 1, c0:c1]

            def nb_(lo, hi, c0, c1):     # -b data (cols 1..W)
                return nbt[:, lo:hi, c0 + 1 : c1 + 1]

            def nbpc(lo, hi, c0, c1):    # -b[y, x-1] (col 0 = zero guard)
                return nbt[:, lo:hi, c0:c1]

            def emit(op_name, lo, hi, out_f, a_f, b_f):
                for eng, c0, c1 in ENG:
                    getattr(eng, op_name)(
                        out_f(lo, hi, c0, c1), a_f(lo, hi, c0, c1),
                        b_f(lo, hi, c0, c1))

            # guard-only memsets (interiors are fully overwritten)
            v.memset(sut[:, :, W:WG], -1.0)
            v.memset(sut[:, NP, :], -1.0)   # partition 127 keeps -1; the
            v.memset(nat[:, 0, :], 0.0)     # boundary DMAs rewrite the rest
            v.memset(nbt[:, :, 0:1], 0.0)
            nc.sync.dma_start(out=eyet[:, :], in_=eye_d)
            nc.sync.dma_start(out=neyet[:, :], in_=neye_d)
            nc.sync.dma_start(out=o2t[:, 0 : NP // 2, :],
                              in_=o2_v[:, 0 : NP // 2, :])
            nc.scalar.dma_start(out=o2t[:, NP // 2 : NP, :],
                                in_=o2_v[:, NP // 2 : NP, :])
            nc.sync.dma_start(out=ng1t[:, :, :], in_=g1_v)
            nc.sync.dma_start(out=ng0t[:, :, :], in_=g0_v)

            def dma_ushift():
                # su[p, 8, x] = t[8p+8, x] = su[p+1, 0, x]; row 127 stays -1
                nc.sync.dma_start(
                    out=sut[0 : P - 1, NP, 0:W], in_=sut[1:P, 0, 0:W]
                )

            def dma_ashift():
                # na[p, 0, x] = -a[8p-1] = na[p-1, 8, x]; row 0 stays 0
                nc.sync.dma_start(
                    out=nat[1:P, 0, 0:W], in_=nat[0 : P - 1, NP, 0:W]
                )

            def mm(ps, w, src, start, stop):
                for h in range(W // 512):
                    pe.matmul(
                        ps[:, h * 512 : (h + 1) * 512],
                        w[:, :],
                        src[:, h * 512 : (h + 1) * 512],
                        start=start,
                        stop=stop,
                    )

            for _rep in range(reps):
                if reps > 1:
                    v.memset(sut[:, 0:NP, 0:W], -1.0)
                for lo, hi in [(0, NP // 2), (NP // 2, NP)]:
                    act.activation(
                        sut[:, lo:hi, 0:W], o2t[:, lo:hi, :], AF.Tanh, scale=0.5
                    )
                dma_ushift()

                for it in range(MAXITER):
                    last = it == MAXITER - 1
                    # --- dual TT: d = shifted diffs, t12 = -g * d ---
                    for lo, hi in BLOCKS:
                        emit("tensor_sub", lo, hi, d1_, unr, u_)
                        emit("tensor_mul", lo, hi, d1_, d1_, ng1_)
                        emit("tensor_sub", lo, hi, d2_, unc, u_)
                        emit("tensor_mul", lo, hi, d2_, d2_, ng0_)
                    # --- dual PE accumulate + relu drain (chunk = 1 plane) ---
                    for i in range(NP):
                        ps1 = psp1.tile([P, W], F32, name="ps1")
                        if it > 0:
                            mm(ps1, eyet, qht[:, i, :], True, False)
                        mm(ps1, eyet, d1t[:, i, :], it == 0, False)
                        mm(ps1, eyet, d2t[:, i, :], False, True)
                        act.activation(qht[:, i, :], ps1[:, :], AF.Relu)
                    # --- primal TT ---
                    # na plane-8 row first so the boundary DMA fires early
                    for eng, c0, c1 in ENG:
                        eng.tensor_mul(
                            nat[:, NP, c0:c1],
                            ng1t[:, NP - 1, c0:c1],
                            qht[:, NP - 1, c0:c1],
                        )
                    dma_ashift()
                    for lo, hi in BLOCKS:
                        hi_w = min(hi, NP - 1)
                        if hi_w > lo:
                            emit("tensor_mul", lo, hi_w, na_, ng1_, qh_)
                        emit("tensor_mul", lo, hi, nb_, ng0_, qh_)
                        # db plane 7 goes via PE double-pass instead (below)
                        hi_db = min(hi, NP - 1)
                        if hi_db > lo:
                            emit("tensor_sub", lo, hi_db, db_, nb_, nbpc)
                    # --- primal PE accumulate + tanh drain / output ---
                    for i in range(NP):
                        ps2 = psp2.tile([P, W], F32, name="ps2")
                        mm(ps2, eyet, o2t[:, i, :], True, False)
                        mm(ps2, eyet, nat[:, i + 1, :], False, False)
                        if i < NP - 1:
                            mm(ps2, neyet, nat[:, i, :], False, False)
                            mm(ps2, eyet, dbt[:, i, :], False, True)
                        else:
                            mm(ps2, neyet, nat[:, i, :], False, False)
                            mm(ps2, eyet, nbt[:, i, 1 : W + 1], False, False)
                            mm(ps2, neyet, nbt[:, i, 0:W], False, True)
                        if last and reps == 1:
                            st = outst[:, i % 4, :]
                            act.activation(st, ps2[:, :], AF.Copy)
                            nc.sync.dma_start(out=out_v[:, i, :], in_=st)
                        else:
                            act.activation(
                                sut[:, i, 0:W], ps2[:, :], AF.Tanh, scale=0.5
                            )
                            if i == 0:
                                dma_ushift()

    nc.compile()
    return nc


def kernel(o, vector_field, nabla_w, div_w):
    global LAST_RESULTS
    if "nc" not in _CACHE:
        _CACHE["nc"] = _build()
    nc = _CACHE["nc"]

    o2 = np.ascontiguousarray(
        (2.0 * np.asarray(o, dtype=np.float32)[:, 0]).astype(np.float16)
    )
    vf = np.asarray(vector_field, dtype=np.float32)
    s = np.float32(-1.0 / np.sqrt(2.0))
    ng0 = np.ascontiguousarray((vf[:, :, 0] * s).astype(np.float16))
    ng1 = np.ascontiguousarray((vf[:, :, 1] * s).astype(np.float16))
    eye = np.eye(P, dtype=np.float16)
    neye = -eye

    in_maps = [
        {"o2": o2[b], "ng0": ng0, "ng1": ng1, "eye": eye, "neye": neye}
        for b in range(B)
    ]
    res = bass_utils.run_bass_kernel_spmd(nc, in_maps, core_ids=list(range(B)))
    LAST_RESULTS = res
    return np.stack([r["out"] for r in res.results]).astype(np.float32)


# revision 16
# speedup vs baseline: 1.0001x; 1.0001x over previous
"""Trainium2 Bass kernel for the Chambolle-Pock-style primal/dual stencil loop.

Math (per image, H=W=1024, EPS=0.5, TAU=0.5, 10 iterations):
    u = sigmoid(o/EPS); q = 0
    repeat 10x:
        q  = relu(q - TAU*(vf1*Dy(u) + vf0*Dx(u)))   # forward diffs, zero pad
        Tq = BDy(vf1*q) + BDx(vf0*q)                  # backward diffs, zero pad
        u  = sigmoid((o - Tq)/EPS)
    return (o - Tq)/EPS

Rescaling: with qh = 2*sqrt(2)*q, g = vf/sqrt(2), o2 = 2*o, s = 2(o - Tq),
t = tanh(s/2)  (u = 0.5 + 0.5*t; zero-padding of u becomes (-1)-padding of t):
    qh = relu(qh - g1*(St - t) - g0*(Rt - t))        # S: y+1 shift, R: x+1
    s  = o2 - (a - Sa) - (b - Rb),  a = g1*qh, b = g0*qh   # backward diffs
    t  = tanh(s/2)
and the final output is s.

Three-engine split (all state fp16; validated rel-L2 vs the fp32 jax
reference ~9e-3, under the 2e-2 gate — the error is early relu
decision-boundary noise, not accumulation):
  - 7 tensor-tensor ops/iter run column-split on DVE (cols 0:839, fp16
    2x_1p mode, 0.52 ns/elem) + GpSimd/Pool (cols 839:1024, TT at 0.42 of
    0.83 ns/elem); the split latency-balances the two engines per op:
      d1 = St - t;  t1 = ng1*d1;  d2 = Rt - t;  t2 = ng0*d2
      na = ng1*qh;  nb = ng0*qh;  db = nb - Rnb          (ng = -g, host-side)
  - the remaining 7 adds/iter run on the otherwise-idle TensorEngine as
    +/-identity matmuls accumulating in PSUM (fp32 accumulation - better
    numerics than fp16 adds), one plane ([128,1024] fp32 = 2 PSUM banks)
    per chunk, double-buffered dual + primal pools = all 8 banks:
      psum1 = I*qh + I*t1 + I*t2                 -> ScalarE relu -> qh (fp16)
      psum2 = I*o2 + I*na - I*Sna + I*db         -> ScalarE tanh(x/2) -> t
    (Sna is the plane-shifted read of na, so the "da" diff costs a PE pass
    instead of a DVE op; db keeps the DVE form except plane 7, which goes
    via an extra PE pass pair - that balance measured best.)
  - last iteration: psum2 chunks are Copy-drained to fp16 and DMA'd out on
    the SP queue (a scalar-queue DMA issue would block ActE's sequencer
    between Copy drains); the host upcasts to fp32.

Boundary handling: image row y = 8*p + i -> partition p (0..127), plane i
(0..7) in the free dim.  Row shifts are free-dim plane offsets; only the
plane7 -> next-partition boundary crosses partitions, via a tiny SBUF->SBUF
DMA per iteration (t: plane 8 of sut; -a: plane 0 of nat, with a dedicated
plane-7 row multiply emitted first so the DMA fires early).  Column shifts
use guard columns (sut col W = -1; nbt col 0 = 0).  Only guard regions are
memset; interiors are fully overwritten (and the iteration-0 dual skips the
I*qh pass since qh0 = 0, so qht needs no init at all).

Sharding: pure data parallel, one image per NeuronCore (B=8 over 8 cores),
ng0/ng1 broadcast.
"""

import numpy as np

import concourse.bacc as bacc
import concourse.mybir as mybir
from concourse.tile import TileContext
from concourse import bass_utils

F32 = mybir.dt.float32
F16 = mybir.dt.float16
AF = mybir.ActivationFunctionType

B, H, W = 8, 1024, 1024
P = 128          # SBUF partitions
NP = H // P      # planes per partition = 8
WG = W + 2       # plane width incl. one guard column (+1 pad to even)
XS = 839         # DVE handles cols [0, XS), Pool cols [XS, W)
NB = 2           # planes per tensor-tensor block
MAXITER = 10

_CACHE = {}
LAST_RESULTS = None  # BassKernelResults of the most recent run (for test.py)


def _build(reps=1):
    nc = bacc.Bacc("TRN2", target_bir_lowering=False, debug=False)

    o2_d = nc.dram_tensor("o2", [H, W], F16, kind="ExternalInput").ap()
    g0_d = nc.dram_tensor("ng0", [H, W], F16, kind="ExternalInput").ap()
    g1_d = nc.dram_tensor("ng1", [H, W], F16, kind="ExternalInput").ap()
    eye_d = nc.dram_tensor("eye", [P, P], F16, kind="ExternalInput").ap()
    neye_d = nc.dram_tensor("neye", [P, P], F16, kind="ExternalInput").ap()
    out_d = nc.dram_tensor("out", [H, W], F16, kind="ExternalOutput").ap()

    # (H, W) -> (p, i, x) with y = 8*p + i
    o2_v = o2_d.rearrange("(p i) x -> p i x", i=NP)
    g0_v = g0_d.rearrange("(p i) x -> p i x", i=NP)
    g1_v = g1_d.rearrange("(p i) x -> p i x", i=NP)
    out_v = out_d.rearrange("(p i) x -> p i x", i=NP)

    v = nc.vector
    gp = nc.gpsimd
    act = nc.scalar
    pe = nc.tensor

    with TileContext(nc) as tc:
        with (
            tc.tile_pool(name="main", bufs=1) as pool,
            tc.tile_pool(name="ps1", bufs=2, space="PSUM") as psp1,
            tc.tile_pool(name="ps2", bufs=2, space="PSUM") as psp2,
        ):
            o2t = pool.tile([P, NP, W], F16)
            ng0t = pool.tile([P, NP, W], F16)
            ng1t = pool.tile([P, NP, W], F16)
            qht = pool.tile([P, NP, W], F16)
            eyet = pool.tile([P, P], F16)
            neyet = pool.tile([P, P], F16)
            # sut: planes 0..7 = t data (col W = -1 guard for x+1 reads),
            # plane 8 = boundary row t[8p+8, x] (partition 127 stays -1)
            sut = pool.tile([P, NP + 1, WG], F16)
            # d1/d2: diff scratch, overwritten in place by t1/t2
            d1t = pool.tile([P, NP, W], F16)
            d2t = pool.tile([P, NP, W], F16)
            # na: planes 1..8 = -a data, plane 0 = boundary row -a[8p-1, x]
            nat = pool.tile([P, NP + 1, W], F16)
            # nb: cols 1..W = -b data, col 0 = zero guard for x-1 reads
            nbt = pool.tile([P, NP, WG], F16)
            dbt = pool.tile([P, NP, W], F16)
            outst = pool.tile([P, 4, W], F16)   # last-iter drain staging

            ENG = [(v, 0, XS), (gp, XS, W)]
            BLOCKS = [(b * NB, (b + 1) * NB) for b in range(NP // NB)]

            def u_(lo, hi, c0, c1):
                return sut[:, lo:hi, c0:c1]

            def unr(lo, hi, c0, c1):   # t[y+1, x] (plane 8 = boundary)
                return sut[:, lo + 1 : hi + 1, c0:c1]

            def unc(lo, hi, c0, c1):   # t[y, x+1] (col W = -1 guard)
                return sut[:, lo:hi, c0 + 1 : c1 + 1]

            def mk(tile):
                def f(lo, hi, c0, c1):
                    return tile[:, lo:hi, c0:c1]
                return f

            o2_, ng0_, ng1_, qh_ = mk(o2t), mk(ng0t), mk(ng1t), mk(qht)
            d1_, d2_, db_ = mk(d1t), mk(d2t), mk(dbt)

            def na_(lo, hi, c0, c1):     # -a data (planes 1..8)
                return nat[:, lo + 1 : hi + 1, c0:c1]

            def nb_(lo, hi, c0, c1):     # -b data (cols 1..W)
                return nbt[:, lo:hi, c0 + 1 : c1 + 1]

            def nbpc(lo, hi, c0, c1):    # -b[y, x-1] (col 0 = zero guard)
                return nbt[:, lo:hi, c0:c1]

            def emit(op_name, lo, hi, out_f, a_f, b_f):
                for eng, c0, c1 in ENG:
                    getattr(eng, op_name)(
                        out_f(lo, hi, c0, c1), a_f(lo, hi, c0, c1),
                        b_f(lo, hi, c0, c1))

            # guard-only memsets (interiors are fully overwritten)
            v.memset(sut[:, :, W:WG], -1.0)
            v.memset(sut[:, NP, :], -1.0)   # partition 127 keeps -1; the
            v.memset(nat[:, 0, :], 0.0)     # boundary DMAs rewrite the rest
            v.memset(nbt[:, :, 0:1], 0.0)
            nc.sync.dma_start(out=eyet[:, :], in_=eye_d)
            nc.sync.dma_start(out=neyet[:, :], in_=neye_d)
            nc.sync.dma_start(out=o2t[:, 0 : NP // 2, :],
                              in_=o2_v[:, 0 : NP // 2, :])
            nc.scalar.dma_start(out=o2t[:, NP // 2 : NP, :],
                                in_=o2_v[:, NP // 2 : NP, :])
            nc.sync.dma_start(out=ng1t[:, :, :], in_=g1_v)
            nc.sync.dma_start(out=ng0t[:, :, :], in_=g0_v)

            def dma_ushift():
                # su[p, 8, x] = t[8p+8, x] = su[p+1, 0, x]; row 127 stays -1
                nc.sync.dma_start(
                    out=sut[0 : P - 1, NP, 0:W], in_=sut[1:P, 0, 0:W]
                )

            def dma_ashift():
                # na[p, 0, x] = -a[8p-1] = na[p-1, 8, x]; row 0 stays 0
                nc.sync.dma_start(
                    out=nat[1:P, 0, 0:W], in_=nat[0 : P - 1, NP, 0:W]
                )

            def mm(ps, w, src, start, stop):
                for h in range(W // 512):
                    pe.matmul(
                        ps[:, h * 512 : (h + 1) * 512],
                        w[:, :],
                        src[:, h * 512 : (h + 1) * 512],
                        start=start,
                        stop=stop,
                    )

            for _rep in range(reps):
                if reps > 1:
                    v.memset(sut[:, 0:NP, 0:W], -1.0)
                for lo, hi in [(0, NP // 2), (NP // 2, NP)]:
                    act.activation(
                        sut[:, lo:hi, 0:W], o2t[:, lo:hi, :], AF.Tanh, scale=0.5
                    )
                dma_ushift()

                for it in range(MAXITER):
                    last = it == MAXITER - 1
                    # --- dual TT: d = shifted diffs, t12 = -g * d ---
                    for lo, hi in BLOCKS:
                        emit("tensor_sub", lo, hi, d1_, unr, u_)
                        emit("tensor_mul", lo, hi, d1_, d1_, ng1_)
                        emit("tensor_sub", lo, hi, d2_, unc, u_)
                        emit("tensor_mul", lo, hi, d2_, d2_, ng0_)
                    # --- dual PE accumulate + relu drain (chunk = 1 plane) ---
                    for i in range(NP):
                        ps1 = psp1.tile([P, W], F32, name="ps1")
                        if it > 0:
                            mm(ps1, eyet, qht[:, i, :], True, False)
                        mm(ps1, eyet, d1t[:, i, :], it == 0, False)
                        mm(ps1, eyet, d2t[:, i, :], False, True)
                        act.activation(qht[:, i, :], ps1[:, :], AF.Relu)
                    # --- primal TT ---
                    # na plane-8 row first so the boundary DMA fires early
                    for eng, c0, c1 in ENG:
                        eng.tensor_mul(
                            nat[:, NP, c0:c1],
                            ng1t[:, NP - 1, c0:c1],
                            qht[:, NP - 1, c0:c1],
                        )
                    dma_ashift()
                    for lo, hi in BLOCKS:
                        hi_w = min(hi, NP - 1)
                        if hi_w > lo:
                            emit("tensor_mul", lo, hi_w, na_, ng1_, qh_)
                        emit("tensor_mul", lo, hi, nb_, ng0_, qh_)
                        # db plane 7 goes via PE double-pass instead (below)
                        hi_db = min(hi, NP - 1)
                        if hi_db > lo:
                            emit("tensor_sub", lo, hi_db, db_, nb_, nbpc)
                    # --- primal PE accumulate + tanh drain / output ---
                    for i in range(NP):
                        ps2 = psp2.tile([P, W], F32, name="ps2")
                        mm(ps2, eyet, o2t[:, i, :], True, False)
                        mm(ps2, eyet, nat[:, i + 1, :], False, False)
                        if i < NP - 1:
                            mm(ps2, neyet, nat[:, i, :], False, False)
                            mm(ps2, eyet, dbt[:, i, :], False, True)
                        else:
                            mm(ps2, neyet, nat[:, i, :], False, False)
                            mm(ps2, eyet, nbt[:, i, 1 : W + 1], False, False)
                            mm(ps2, neyet, nbt[:, i, 0:W], False, True)
                        if last and reps == 1:
                            st = outst[:, i % 4, :]
                            act.activation(st, ps2[:, :], AF.Copy)
                            nc.sync.dma_start(out=out_v[:, i, :], in_=st)
                        else:
                            act.activation(
                                sut[:, i, 0:W], ps2[:, :], AF.Tanh, scale=0.5
                            )
                            if i == 0:
                                dma_ushift()

    nc.compile()
    return nc


def kernel(o, vector_field, nabla_w, div_w):
    global LAST_RESULTS
    if "nc" not in _CACHE:
        _CACHE["nc"] = _build()
    nc = _CACHE["nc"]

    o2 = np.ascontiguousarray(
        (2.0 * np.asarray(o, dtype=np.float32)[:, 0]).astype(np.float16)
    )
    vf = np.asarray(vector_field, dtype=np.float32)
    s = np.float32(-1.0 / np.sqrt(2.0))
    ng0 = np.ascontiguousarray((vf[:, :, 0] * s).astype(np.float16))
    ng1 = np.ascontiguousarray((vf[:, :, 1] * s).astype(np.float16))
    eye = np.eye(P, dtype=np.float16)
    neye = -eye

    in_maps = [
        {"o2": o2[b], "ng0": ng0, "ng1": ng1, "eye": eye, "neye": neye}
        for b in range(B)
    ]
    res = bass_utils.run_bass_kernel_spmd(nc, in_maps, core_ids=list(range(B)))
    LAST_RESULTS = res
    return np.stack([r["out"] for r in res.results]).astype(np.float32)


# revision 17
# speedup vs baseline: 1.0114x; 1.0113x over previous
"""Trainium2 Bass kernel for the Chambolle-Pock-style primal/dual stencil loop.

Math (per image, H=W=1024, EPS=0.5, TAU=0.5, 10 iterations):
    u = sigmoid(o/EPS); q = 0
    repeat 10x:
        q  = relu(q - TAU*(vf1*Dy(u) + vf0*Dx(u)))   # forward diffs, zero pad
        Tq = BDy(vf1*q) + BDx(vf0*q)                  # backward diffs, zero pad
        u  = sigmoid((o - Tq)/EPS)
    return (o - Tq)/EPS

Rescaling: with qh = 2*sqrt(2)*q, g = vf/sqrt(2), o2 = 2*o, s = 2(o - Tq),
t = tanh(s/2)  (u = 0.5 + 0.5*t; zero-padding of u becomes (-1)-padding of t):
    qh = relu(qh - g1*(St - t) - g0*(Rt - t))        # S: y+1 shift, R: x+1
    s  = o2 - (a - Sa) - (b - Rb),  a = g1*qh, b = g0*qh   # backward diffs
    t  = tanh(s/2)
and the final output is s.

Three-engine split (all state fp16; validated rel-L2 vs the fp32 jax
reference ~9e-3, under the 2e-2 gate — the error is early relu
decision-boundary noise, not accumulation):
  - 7 tensor-tensor ops/iter run column-split on DVE (cols 0:829, fp16
    2x_1p mode, 0.52 ns/elem) + GpSimd/Pool (cols 829:1024, TT at 0.42 of
    0.83 ns/elem); the split latency-balances the two engines per op:
      d1 = St - t;  t1 = ng1*d1;  d2 = Rt - t;  t2 = ng0*d2
      na = ng1*qh;  nb = ng0*qh;  db = nb - Rnb          (ng = -g, host-side)
  - the remaining 7 adds/iter run on the otherwise-idle TensorEngine as
    +/-identity matmuls accumulating in PSUM (fp32 accumulation - better
    numerics than fp16 adds), one plane ([128,1024] fp32 = 2 PSUM banks)
    per chunk, one shared 4-deep psum pool = all 8 banks:
      psum1 = I*qh + I*t1 + I*t2                 -> ScalarE relu -> qh (fp16)
      psum2 = I*o2 + I*na - I*Sna + I*db         -> ScalarE tanh(x/2) -> t
    (Sna is the plane-shifted read of na, so the "da" diff costs a PE pass
    instead of a DVE op; db keeps the DVE form except plane 7, which goes
    via an extra PE pass pair - that balance measured best.)
  - last iteration: psum2 chunks are Copy-drained to fp16 and DMA'd out on
    the SP queue (a scalar-queue DMA issue would block ActE's sequencer
    between Copy drains); the host upcasts to fp32.

Boundary handling: image row y = 8*p + i -> partition p (0..127), plane i
(0..7) in the free dim.  Row shifts are free-dim plane offsets; only the
plane7 -> next-partition boundary crosses partitions, via a tiny SBUF->SBUF
DMA per iteration (t: plane 8 of sut; -a: plane 0 of nat, with a dedicated
plane-7 row multiply emitted first so the DMA fires early).  Column shifts
use guard columns (sut col W = -1; nbt col 0 = 0).  Only guard regions are
memset; interiors are fully overwritten (and the iteration-0 dual skips the
I*qh pass since qh0 = 0, so qht needs no init at all).

Sharding: pure data parallel, one image per NeuronCore (B=8 over 8 cores),
ng0/ng1 broadcast.
"""

import numpy as np

import concourse.bacc as bacc
import concourse.mybir as mybir
from concourse.tile import TileContext
from concourse import bass_utils

F32 = mybir.dt.float32
F16 = mybir.dt.float16
AF = mybir.ActivationFunctionType

B, H, W = 8, 1024, 1024
P = 128          # SBUF partitions
NP = H // P      # planes per partition = 8
WG = W + 2       # plane width incl. one guard column (+1 pad to even)
XS = 829         # DVE handles cols [0, XS), Pool cols [XS, W)
NB = 2           # planes per tensor-tensor block
MAXITER = 10

_CACHE = {}
LAST_RESULTS = None  # BassKernelResults of the most recent run (for test.py)


def _build(reps=1):
    nc = bacc.Bacc("TRN2", target_bir_lowering=False, debug=False)

    o2_d = nc.dram_tensor("o2", [H, W], F16, kind="ExternalInput").ap()
    g0_d = nc.dram_tensor("ng0", [H, W], F16, kind="ExternalInput").ap()
    g1_d = nc.dram_tensor("ng1", [H, W], F16, kind="ExternalInput").ap()
    eye_d = nc.dram_tensor("eye", [P, P], F16, kind="ExternalInput").ap()
    neye_d = nc.dram_tensor("neye", [P, P], F16, kind="ExternalInput").ap()
    out_d = nc.dram_tensor("out", [H, W], F16, kind="ExternalOutput").ap()

    # (H, W) -> (p, i, x) with y = 8*p + i
    o2_v = o2_d.rearrange("(p i) x -> p i x", i=NP)
    g0_v = g0_d.rearrange("(p i) x -> p i x", i=NP)
    g1_v = g1_d.rearrange("(p i) x -> p i x", i=NP)
    out_v = out_d.rearrange("(p i) x -> p i x", i=NP)

    v = nc.vector
    gp = nc.gpsimd
    act = nc.scalar
    pe = nc.tensor

    with TileContext(nc) as tc:
        with (
            tc.tile_pool(name="main", bufs=1) as pool,
            # one SHARED psum pool, 4-deep rotation across dual+primal
            # chunks: doubles the in-phase lookahead vs two 2-buf pools
            # (same 8 banks) and unlocks a more Pool-shifted column split
            tc.tile_pool(name="psp", bufs=4, space="PSUM") as psp,
        ):
            o2t = pool.tile([P, NP, W], F16)
            ng0t = pool.tile([P, NP, W], F16)
            ng1t = pool.tile([P, NP, W], F16)
            qht = pool.tile([P, NP, W], F16)
            eyet = pool.tile([P, P], F16)
            neyet = pool.tile([P, P], F16)
            # sut: planes 0..7 = t data (col W = -1 guard for x+1 reads),
            # plane 8 = boundary row t[8p+8, x] (partition 127 stays -1)
            sut = pool.tile([P, NP + 1, WG], F16)
            # d1/d2: diff scratch, overwritten in place by t1/t2
            d1t = pool.tile([P, NP, W], F16)
            d2t = pool.tile([P, NP, W], F16)
            # na: planes 1..8 = -a data, plane 0 = boundary row -a[8p-1, x]
            nat = pool.tile([P, NP + 1, W], F16)
            # nb: cols 1..W = -b data, col 0 = zero guard for x-1 reads
            nbt = pool.tile([P, NP, WG], F16)
            dbt = pool.tile([P, NP, W], F16)
            outst = pool.tile([P, 4, W], F16)   # last-iter drain staging

            ENG = [(v, 0, XS), (gp, XS, W)]
            BLOCKS = [(b * NB, (b + 1) * NB) for b in range(NP // NB)]

            def u_(lo, hi, c0, c1):
                return sut[:, lo:hi, c0:c1]

            def unr(lo, hi, c0, c1):   # t[y+1, x] (plane 8 = boundary)
                return sut[:, lo + 1 : hi + 1, c0:c1]

            def unc(lo, hi, c0, c1):   # t[y, x+1] (col W = -1 guard)
                return sut[:, lo:hi, c0 + 1 : c1 + 1]

            def mk(tile):
                def f(lo, hi, c0, c1):
                    return tile[:, lo:hi, c0:c1]
                return f

            o2_, ng0_, ng1_, qh_ = mk(o2t), mk(ng0t), mk(ng1t), mk(qht)
            d1_, d2_, db_ = mk(d1t), mk(d2t), mk(dbt)

            def na_(lo, hi, c0, c1):     # -a data (planes 1..8)
                return nat[:, lo + 1 : hi + 1, c0:c1]

            def nb_(lo, hi, c0, c1):     # -b data (cols 1..W)
                return nbt[:, lo:hi, c0 + 1 : c1 + 1]

            def nbpc(lo, hi, c0, c1):    # -b[y, x-1] (col 0 = zero guard)
                return nbt[:, lo:hi, c0:c1]

            def emit(op_name, lo, hi, out_f, a_f, b_f):
                for eng, c0, c1 in ENG:
                    getattr(eng, op_name)(
                        out_f(lo, hi, c0, c1), a_f(lo, hi, c0, c1),
                        b_f(lo, hi, c0, c1))

            # guard-only memsets (interiors are fully overwritten)
            v.memset(sut[:, :, W:WG], -1.0)
            v.memset(sut[:, NP, :], -1.0)   # partition 127 keeps -1; the
            v.memset(nat[:, 0, :], 0.0)     # boundary DMAs rewrite the rest
            v.memset(nbt[:, :, 0:1], 0.0)
            nc.sync.dma_start(out=eyet[:, :], in_=eye_d)
            nc.sync.dma_start(out=neyet[:, :], in_=neye_d)
            nc.sync.dma_start(out=o2t[:, 0 : NP // 2, :],
                              in_=o2_v[:, 0 : NP // 2, :])
            nc.scalar.dma_start(out=o2t[:, NP // 2 : NP, :],
                                in_=o2_v[:, NP // 2 : NP, :])
            nc.sync.dma_start(out=ng1t[:, :, :], in_=g1_v)
            nc.sync.dma_start(out=ng0t[:, :, :], in_=g0_v)

            def dma_ushift():
                # su[p, 8, x] = t[8p+8, x] = su[p+1, 0, x]; row 127 stays -1
                nc.sync.dma_start(
                    out=sut[0 : P - 1, NP, 0:W], in_=sut[1:P, 0, 0:W]
                )

            def dma_ashift():
                # na[p, 0, x] = -a[8p-1] = na[p-1, 8, x]; row 0 stays 0
                nc.sync.dma_start(
                    out=nat[1:P, 0, 0:W], in_=nat[0 : P - 1, NP, 0:W]
                )

            def mm(ps, w, src, start, stop):
                for h in range(W // 512):
                    pe.matmul(
                        ps[:, h * 512 : (h + 1) * 512],
                        w[:, :],
                        src[:, h * 512 : (h + 1) * 512],
                        start=start,
                        stop=stop,
                    )

            for _rep in range(reps):
                if reps > 1:
                    v.memset(sut[:, 0:NP, 0:W], -1.0)
                for lo, hi in [(0, NP // 2), (NP // 2, NP)]:
                    act.activation(
                        sut[:, lo:hi, 0:W], o2t[:, lo:hi, :], AF.Tanh, scale=0.5
                    )
                dma_ushift()

                for it in range(MAXITER):
                    last = it == MAXITER - 1
                    # --- dual TT: d = shifted diffs, t12 = -g * d ---
                    for lo, hi in BLOCKS:
                        emit("tensor_sub", lo, hi, d1_, unr, u_)
                        emit("tensor_mul", lo, hi, d1_, d1_, ng1_)
                        emit("tensor_sub", lo, hi, d2_, unc, u_)
                        emit("tensor_mul", lo, hi, d2_, d2_, ng0_)
                    # --- dual PE accumulate + relu drain (chunk = 1 plane) ---
                    for i in range(NP):
                        ps1 = psp.tile([P, W], F32, name="ps")
                        if it > 0:
                            mm(ps1, eyet, qht[:, i, :], True, False)
                        mm(ps1, eyet, d1t[:, i, :], it == 0, False)
                        mm(ps1, eyet, d2t[:, i, :], False, True)
                        act.activation(qht[:, i, :], ps1[:, :], AF.Relu)
                    # --- primal TT ---
                    # na plane-8 row first so the boundary DMA fires early
                    for eng, c0, c1 in ENG:
                        eng.tensor_mul(
                            nat[:, NP, c0:c1],
                            ng1t[:, NP - 1, c0:c1],
                            qht[:, NP - 1, c0:c1],
                        )
                    dma_ashift()
                    for lo, hi in BLOCKS:
                        hi_w = min(hi, NP - 1)
                        if hi_w > lo:
                            emit("tensor_mul", lo, hi_w, na_, ng1_, qh_)
                        emit("tensor_mul", lo, hi, nb_, ng0_, qh_)
                        # db plane 7 goes via PE double-pass instead (below)
                        hi_db = min(hi, NP - 1)
                        if hi_db > lo:
                            emit("tensor_sub", lo, hi_db, db_, nb_, nbpc)
                    # --- primal PE accumulate + tanh drain / output ---
                    for i in range(NP):
                        ps2 = psp.tile([P, W], F32, name="ps")
                        mm(ps2, eyet, o2t[:, i, :], True, False)
                        mm(ps2, eyet, nat[:, i + 1, :], False, False)
                        if i < NP - 1:
                            mm(ps2, neyet, nat[:, i, :], False, False)
                            mm(ps2, eyet, dbt[:, i, :], False, True)
                        else:
                            mm(ps2, neyet, nat[:, i, :], False, False)
                            mm(ps2, eyet, nbt[:, i, 1 : W + 1], False, False)
                            mm(ps2, neyet, nbt[:, i, 0:W], False, True)
                        if last and reps == 1:
                            st = outst[:, i % 4, :]
                            act.activation(st, ps2[:, :], AF.Copy)
                            nc.sync.dma_start(out=out_v[:, i, :], in_=st)
                        else:
                            act.activation(
                                sut[:, i, 0:W], ps2[:, :], AF.Tanh, scale=0.5
                            )
                            if i == 0:
                                dma_ushift()

    nc.compile()
    return nc


def kernel(o, vector_field, nabla_w, div_w):
    global LAST_RESULTS
    if "nc" not in _CACHE:
        _CACHE["nc"] = _build()
    nc = _CACHE["nc"]

    o2 = np.ascontiguousarray(
        (2.0 * np.asarray(o, dtype=np.float32)[:, 0]).astype(np.float16)
    )
    vf = np.asarray(vector_field, dtype=np.float32)
    s = np.float32(-1.0 / np.sqrt(2.0))
    ng0 = np.ascontiguousarray((vf[:, :, 0] * s).astype(np.float16))
    ng1 = np.ascontiguousarray((vf[:, :, 1] * s).astype(np.float16))
    eye = np.eye(P, dtype=np.float16)
    neye = -eye

    in_maps = [
        {"o2": o2[b], "ng0": ng0, "ng1": ng1, "eye": eye, "neye": neye}
        for b in range(B)
    ]
    res = bass_utils.run_bass_kernel_spmd(nc, in_maps, core_ids=list(range(B)))
    LAST_RESULTS = res
    return np.stack([r["out"] for r in res.results]).astype(np.float32)


# revision 18
# speedup vs baseline: 1.0267x; 1.0151x over previous
"""Trainium2 Bass kernel for the Chambolle-Pock-style primal/dual stencil loop.

Math (per image, H=W=1024, EPS=0.5, TAU=0.5, 10 iterations):
    u = sigmoid(o/EPS); q = 0
    repeat 10x:
        q  = relu(q - TAU*(vf1*Dy(u) + vf0*Dx(u)))   # forward diffs, zero pad
        Tq = BDy(vf1*q) + BDx(vf0*q)                  # backward diffs, zero pad
        u  = sigmoid((o - Tq)/EPS)
    return (o - Tq)/EPS

Rescaling: with qh = 2*sqrt(2)*q, g = vf/sqrt(2), o2 = 2*o, s = 2(o - Tq),
t = tanh(s/2)  (u = 0.5 + 0.5*t; zero-padding of u becomes (-1)-padding of t):
    qh = relu(qh - g1*(St - t) - g0*(Rt - t))        # S: y+1 shift, R: x+1
    s  = o2 - (a - Sa) - (b - Rb),  a = g1*qh, b = g0*qh   # backward diffs
    t  = tanh(s/2)
and the final output is s.

Three-engine split (all state fp16; validated rel-L2 vs the fp32 jax
reference ~9e-3, under the 2e-2 gate — the error is early relu
decision-boundary noise, not accumulation):
  - 7 tensor-tensor ops/iter run column-split on DVE (cols 0:832, fp16
    2x_1p mode, 0.52 ns/elem) + GpSimd/Pool (cols 832:1024, TT at 0.42 of
    0.83 ns/elem); the split latency-balances the two engines per op:
      d1 = St - t;  t1 = ng1*d1;  d2 = Rt - t;  t2 = ng0*d2
      na = ng1*qh;  nb = ng0*qh;  db = nb - Rnb          (ng = -g, host-side)
  - the remaining 7 adds/iter run on the otherwise-idle TensorEngine as
    +/-identity matmuls accumulating in PSUM (fp32 accumulation - better
    numerics than fp16 adds), one half-plane ([128,512] fp32 = 1 PSUM
    bank) per chunk, one shared 8-deep psum pool = all 8 banks:
      psum1 = I*qh + I*t1 + I*t2                 -> ScalarE relu -> qh (fp16)
      psum2 = I*o2 + I*na - I*Sna + I*db         -> ScalarE tanh(x/2) -> t
    (Sna is the plane-shifted read of na, so the "da" diff costs a PE pass
    instead of a DVE op; db keeps the DVE form except plane 7, which goes
    via an extra PE pass pair - that balance measured best.)
  - last iteration: psum2 chunks are Copy-drained to fp16 and DMA'd out on
    the SP queue (a scalar-queue DMA issue would block ActE's sequencer
    between Copy drains); the host upcasts to fp32.

Boundary handling: image row y = 8*p + i -> partition p (0..127), plane i
(0..7) in the free dim.  Row shifts are free-dim plane offsets; only the
plane7 -> next-partition boundary crosses partitions, via a tiny SBUF->SBUF
DMA per iteration (t: plane 8 of sut; -a: plane 0 of nat, with a dedicated
plane-7 row multiply emitted first so the DMA fires early).  Column shifts
use guard columns (sut col W = -1; nbt col 0 = 0).  Only guard regions are
memset; interiors are fully overwritten (and the iteration-0 dual skips the
I*qh pass since qh0 = 0, so qht needs no init at all).

Sharding: pure data parallel, one image per NeuronCore (B=8 over 8 cores),
ng0/ng1 broadcast.
"""

import numpy as np

import concourse.bacc as bacc
import concourse.mybir as mybir
from concourse.tile import TileContext
from concourse import bass_utils

F32 = mybir.dt.float32
F16 = mybir.dt.float16
AF = mybir.ActivationFunctionType

B, H, W = 8, 1024, 1024
P = 128          # SBUF partitions
NP = H // P      # planes per partition = 8
WG = W + 2       # plane width incl. one guard column (+1 pad to even)
XS = 832         # DVE handles cols [0, XS), Pool cols [XS, W)
NB = 2           # planes per tensor-tensor block
MAXITER = 10

_CACHE = {}
LAST_RESULTS = None  # BassKernelResults of the most recent run (for test.py)


def _build(reps=1):
    nc = bacc.Bacc("TRN2", target_bir_lowering=False, debug=False)

    o2_d = nc.dram_tensor("o2", [H, W], F16, kind="ExternalInput").ap()
    g0_d = nc.dram_tensor("ng0", [H, W], F16, kind="ExternalInput").ap()
    g1_d = nc.dram_tensor("ng1", [H, W], F16, kind="ExternalInput").ap()
    eye_d = nc.dram_tensor("eye", [P, P], F16, kind="ExternalInput").ap()
    neye_d = nc.dram_tensor("neye", [P, P], F16, kind="ExternalInput").ap()
    out_d = nc.dram_tensor("out", [H, W], F16, kind="ExternalOutput").ap()

    # (H, W) -> (p, i, x) with y = 8*p + i
    o2_v = o2_d.rearrange("(p i) x -> p i x", i=NP)
    g0_v = g0_d.rearrange("(p i) x -> p i x", i=NP)
    g1_v = g1_d.rearrange("(p i) x -> p i x", i=NP)
    out_v = out_d.rearrange("(p i) x -> p i x", i=NP)

    v = nc.vector
    gp = nc.gpsimd
    act = nc.scalar
    pe = nc.tensor

    with TileContext(nc) as tc:
        with (
            tc.tile_pool(name="main", bufs=1) as pool,
            # one SHARED psum pool of half-plane chunks ([128,512] fp32 =
            # 1 bank), 8-deep rotation across dual+primal: maximum lookahead
            # on the same 8 banks; unlocks a more Pool-shifted column split
            tc.tile_pool(name="psp", bufs=8, space="PSUM") as psp,
        ):
            o2t = pool.tile([P, NP, W], F16)
            ng0t = pool.tile([P, NP, W], F16)
            ng1t = pool.tile([P, NP, W], F16)
            qht = pool.tile([P, NP, W], F16)
            eyet = pool.tile([P, P], F16)
            neyet = pool.tile([P, P], F16)
            # sut: planes 0..7 = t data (col W = -1 guard for x+1 reads),
            # plane 8 = boundary row t[8p+8, x] (partition 127 stays -1)
            sut = pool.tile([P, NP + 1, WG], F16)
            # d1/d2: diff scratch, overwritten in place by t1/t2
            d1t = pool.tile([P, NP, W], F16)
            d2t = pool.tile([P, NP, W], F16)
            # na: planes 1..8 = -a data, plane 0 = boundary row -a[8p-1, x]
            nat = pool.tile([P, NP + 1, W], F16)
            # nb: cols 1..W = -b data, col 0 = zero guard for x-1 reads
            nbt = pool.tile([P, NP, WG], F16)
            dbt = pool.tile([P, NP, W], F16)
            outst = pool.tile([P, 4, W], F16)   # last-iter drain staging

            ENG = [(v, 0, XS), (gp, XS, W)]
            BLOCKS = [(b * NB, (b + 1) * NB) for b in range(NP // NB)]

            def u_(lo, hi, c0, c1):
                return sut[:, lo:hi, c0:c1]

            def unr(lo, hi, c0, c1):   # t[y+1, x] (plane 8 = boundary)
                return sut[:, lo + 1 : hi + 1, c0:c1]

            def unc(lo, hi, c0, c1):   # t[y, x+1] (col W = -1 guard)
                return sut[:, lo:hi, c0 + 1 : c1 + 1]

            def mk(tile):
                def f(lo, hi, c0, c1):
                    return tile[:, lo:hi, c0:c1]
                return f

            o2_, ng0_, ng1_, qh_ = mk(o2t), mk(ng0t), mk(ng1t), mk(qht)
            d1_, d2_, db_ = mk(d1t), mk(d2t), mk(dbt)

            def na_(lo, hi, c0, c1):     # -a data (planes 1..8)
                return nat[:, lo + 1 : hi + 1, c0:c1]

            def nb_(lo, hi, c0, c1):     # -b data (cols 1..W)
                return nbt[:, lo:hi, c0 + 1 : c1 + 1]

            def nbpc(lo, hi, c0, c1):    # -b[y, x-1] (col 0 = zero guard)
                return nbt[:, lo:hi, c0:c1]

            def emit(op_name, lo, hi, out_f, a_f, b_f):
                for eng, c0, c1 in ENG:
                    getattr(eng, op_name)(
                        out_f(lo, hi, c0, c1), a_f(lo, hi, c0, c1),
                        b_f(lo, hi, c0, c1))

            # guard-only memsets (interiors are fully overwritten)
            v.memset(sut[:, :, W:WG], -1.0)
            v.memset(sut[:, NP, :], -1.0)   # partition 127 keeps -1; the
            v.memset(nat[:, 0, :], 0.0)     # boundary DMAs rewrite the rest
            v.memset(nbt[:, :, 0:1], 0.0)
            nc.sync.dma_start(out=eyet[:, :], in_=eye_d)
            nc.sync.dma_start(out=neyet[:, :], in_=neye_d)
            nc.sync.dma_start(out=o2t[:, 0 : NP // 2, :],
                              in_=o2_v[:, 0 : NP // 2, :])
            nc.scalar.dma_start(out=o2t[:, NP // 2 : NP, :],
                                in_=o2_v[:, NP // 2 : NP, :])
            nc.sync.dma_start(out=ng1t[:, :, :], in_=g1_v)
            nc.sync.dma_start(out=ng0t[:, :, :], in_=g0_v)

            def dma_ushift():
                # su[p, 8, x] = t[8p+8, x] = su[p+1, 0, x]; row 127 stays -1
                nc.sync.dma_start(
                    out=sut[0 : P - 1, NP, 0:W], in_=sut[1:P, 0, 0:W]
                )

            def dma_ashift():
                # na[p, 0, x] = -a[8p-1] = na[p-1, 8, x]; row 0 stays 0
                nc.sync.dma_start(
                    out=nat[1:P, 0, 0:W], in_=nat[0 : P - 1, NP, 0:W]
                )

            for _rep in range(reps):
                if reps > 1:
                    v.memset(sut[:, 0:NP, 0:W], -1.0)
                for lo, hi in [(0, NP // 2), (NP // 2, NP)]:
                    act.activation(
                        sut[:, lo:hi, 0:W], o2t[:, lo:hi, :], AF.Tanh, scale=0.5
                    )
                dma_ushift()

                for it in range(MAXITER):
                    last = it == MAXITER - 1
                    # --- dual TT: d = shifted diffs, t12 = -g * d ---
                    for lo, hi in BLOCKS:
                        emit("tensor_sub", lo, hi, d1_, unr, u_)
                        emit("tensor_mul", lo, hi, d1_, d1_, ng1_)
                        emit("tensor_sub", lo, hi, d2_, unc, u_)
                        emit("tensor_mul", lo, hi, d2_, d2_, ng0_)
                    # --- dual PE accumulate + relu drain (half-plane) ---
                    for i in range(NP):
                      for hc0, hc1 in [(0, 512), (512, W)]:
                        ps1 = psp.tile([P, 512], F32, name="ps")
                        if it > 0:
                            pe.matmul(ps1[:, :], eyet[:, :],
                                      qht[:, i, hc0:hc1], start=True,
                                      stop=False)
                        pe.matmul(ps1[:, :], eyet[:, :], d1t[:, i, hc0:hc1],
                                  start=(it == 0), stop=False)
                        pe.matmul(ps1[:, :], eyet[:, :], d2t[:, i, hc0:hc1],
                                  start=False, stop=True)
                        act.activation(qht[:, i, hc0:hc1], ps1[:, :], AF.Relu)
                    # --- primal TT ---
                    # na plane-8 row first so the boundary DMA fires early
                    for eng, c0, c1 in ENG:
                        eng.tensor_mul(
                            nat[:, NP, c0:c1],
                            ng1t[:, NP - 1, c0:c1],
                            qht[:, NP - 1, c0:c1],
                        )
                    dma_ashift()
                    for lo, hi in BLOCKS:
                        hi_w = min(hi, NP - 1)
                        if hi_w > lo:
                            emit("tensor_mul", lo, hi_w, na_, ng1_, qh_)
                        emit("tensor_mul", lo, hi, nb_, ng0_, qh_)
                        # db plane 7 goes via PE double-pass instead (below)
                        hi_db = min(hi, NP - 1)
                        if hi_db > lo:
                            emit("tensor_sub", lo, hi_db, db_, nb_, nbpc)
                    # --- primal PE accumulate + tanh drain / output ---
                    for i in range(NP):
                      for hc0, hc1 in [(0, 512), (512, W)]:
                        ps2 = psp.tile([P, 512], F32, name="ps")
                        pe.matmul(ps2[:, :], eyet[:, :], o2t[:, i, hc0:hc1],
                                  start=True, stop=False)
                        pe.matmul(ps2[:, :], eyet[:, :],
                                  nat[:, i + 1, hc0:hc1], start=False,
                                  stop=False)
                        pe.matmul(ps2[:, :], neyet[:, :], nat[:, i, hc0:hc1],
                                  start=False, stop=False)
                        if i < NP - 1:
                            pe.matmul(ps2[:, :], eyet[:, :],
                                      dbt[:, i, hc0:hc1], start=False,
                                      stop=True)
                        else:
                            pe.matmul(ps2[:, :], eyet[:, :],
                                      nbt[:, i, 1 + hc0 : 1 + hc1],
                                      start=False, stop=False)
                            pe.matmul(ps2[:, :], neyet[:, :],
                                      nbt[:, i, hc0:hc1], start=False,
                                      stop=True)
                        if last and reps == 1:
                            st = outst[:, i % 4, hc0:hc1]
                            act.activation(st, ps2[:, :], AF.Copy)
                            nc.sync.dma_start(out=out_v[:, i, hc0:hc1],
                                              in_=st)
                        else:
                            act.activation(
                                sut[:, i, hc0:hc1], ps2[:, :], AF.Tanh,
                                scale=0.5,
                            )
                            if i == 0 and hc1 == W:
                                dma_ushift()

    nc.compile()
    return nc


def kernel(o, vector_field, nabla_w, div_w):
    global LAST_RESULTS
    if "nc" not in _CACHE:
        _CACHE["nc"] = _build()
    nc = _CACHE["nc"]

    o2 = np.ascontiguousarray(
        (2.0 * np.asarray(o, dtype=np.float32)[:, 0]).astype(np.float16)
    )
    vf = np.asarray(vector_field, dtype=np.float32)
    s = np.float32(-1.0 / np.sqrt(2.0))
    ng0 = np.ascontiguousarray((vf[:, :, 0] * s).astype(np.float16))
    ng1 = np.ascontiguousarray((vf[:, :, 1] * s).astype(np.float16))
    eye = np.eye(P, dtype=np.float16)
    neye = -eye

    in_maps = [
        {"o2": o2[b], "ng0": ng0, "ng1": ng1, "eye": eye, "neye": neye}
        for b in range(B)
    ]
    res = bass_utils.run_bass_kernel_spmd(nc, in_maps, core_ids=list(range(B)))
    LAST_RESULTS = res
    return np.stack([r["out"] for r in res.results]).astype(np.float32)


# revision 19
# speedup vs baseline: 1.0346x; 1.0076x over previous
"""Trainium2 Bass kernel for the Chambolle-Pock-style primal/dual stencil loop.

Math (per image, H=W=1024, EPS=0.5, TAU=0.5, 10 iterations):
    u = sigmoid(o/EPS); q = 0
    repeat 10x:
        q  = relu(q - TAU*(vf1*Dy(u) + vf0*Dx(u)))   # forward diffs, zero pad
        Tq = BDy(vf1*q) + BDx(vf0*q)                  # backward diffs, zero pad
        u  = sigmoid((o - Tq)/EPS)
    return (o - Tq)/EPS

Rescaling: with qh = 2*sqrt(2)*q, g = vf/sqrt(2), o2 = 2*o, s = 2(o - Tq),
t = tanh(s/2)  (u = 0.5 + 0.5*t; zero-padding of u becomes (-1)-padding of t):
    qh = relu(qh - g1*(St - t) - g0*(Rt - t))        # S: y+1 shift, R: x+1
    s  = o2 - (a - Sa) - (b - Rb),  a = g1*qh, b = g0*qh   # backward diffs
    t  = tanh(s/2)
and the final output is s.

Three-engine split (all state fp16; validated rel-L2 vs the fp32 jax
reference ~9e-3, under the 2e-2 gate — the error is early relu
decision-boundary noise, not accumulation):
  - 7 tensor-tensor ops/iter run column-split on DVE (cols 0:832, fp16
    2x_1p mode, 0.52 ns/elem) + GpSimd/Pool (cols 832:1024, TT at 0.42 of
    0.83 ns/elem); the split latency-balances the two engines per op:
      d1 = St - t;  t1 = ng1*d1;  d2 = Rt - t;  t2 = ng0*d2
      na = ng1*qh;  nb = ng0*qh;  db = nb - Rnb          (ng = -g, host-side)
  - the remaining 7 adds/iter run on the otherwise-idle TensorEngine as
    +/-identity matmuls accumulating in PSUM (fp32 accumulation - better
    numerics than fp16 adds), one half-plane ([128,512] fp32 = 1 PSUM
    bank) per chunk, one shared 8-deep psum pool = all 8 banks:
      psum1 = I*qh + I*t1 + I*t2                 -> ScalarE relu -> qh (fp16)
      psum2 = I*o2 + I*na - I*Sna + I*db         -> ScalarE tanh(x/2) -> t
    (Sna is the plane-shifted read of na, so the "da" diff costs a PE pass
    instead of a DVE op; db keeps the DVE form except plane 7, which goes
    via an extra PE pass pair - that balance measured best.)
  - last iteration: psum2 chunks are Copy-drained to fp16 and DMA'd out on
    the SP queue (a scalar-queue DMA issue would block ActE's sequencer
    between Copy drains); the host upcasts to fp32.

Boundary handling: image row y = 8*p + i -> partition p (0..127), plane i
(0..7) in the free dim.  Row shifts are free-dim plane offsets; only the
plane7 -> next-partition boundary crosses partitions, via a tiny SBUF->SBUF
DMA per iteration (t: plane 8 of sut; -a: plane 0 of nat, with a dedicated
plane-7 row multiply emitted first so the DMA fires early).  Column shifts
use guard columns (sut col W = -1; nbt col 0 = 0).  Only guard regions are
memset; interiors are fully overwritten (and the iteration-0 dual skips the
I*qh pass since qh0 = 0, so qht needs no init at all).

Sharding: pure data parallel, one image per NeuronCore (B=8 over 8 cores),
ng0/ng1 broadcast.
"""

import numpy as np

import concourse.bacc as bacc
import concourse.mybir as mybir
from concourse.tile import TileContext
from concourse import bass_utils

F32 = mybir.dt.float32
F16 = mybir.dt.float16
AF = mybir.ActivationFunctionType

B, H, W = 8, 1024, 1024
P = 128          # SBUF partitions
NP = H // P      # planes per partition = 8
WG = W + 2       # plane width incl. one guard column (+1 pad to even)
XS = 832         # DVE handles cols [0, XS), Pool cols [XS, W)
NB = 2           # planes per tensor-tensor block
MAXITER = 10

_CACHE = {}
LAST_RESULTS = None  # BassKernelResults of the most recent run (for test.py)


def _build(reps=1):
    nc = bacc.Bacc("TRN2", target_bir_lowering=False, debug=False)

    o2_d = nc.dram_tensor("o2", [H, W], F16, kind="ExternalInput").ap()
    g0_d = nc.dram_tensor("ng0", [H, W], F16, kind="ExternalInput").ap()
    g1_d = nc.dram_tensor("ng1", [H, W], F16, kind="ExternalInput").ap()
    eye_d = nc.dram_tensor("eye", [P, P], F16, kind="ExternalInput").ap()
    neye_d = nc.dram_tensor("neye", [P, P], F16, kind="ExternalInput").ap()
    out_d = nc.dram_tensor("out", [H, W], F16, kind="ExternalOutput").ap()

    # (H, W) -> (p, i, x) with y = 8*p + i
    o2_v = o2_d.rearrange("(p i) x -> p i x", i=NP)
    g0_v = g0_d.rearrange("(p i) x -> p i x", i=NP)
    g1_v = g1_d.rearrange("(p i) x -> p i x", i=NP)
    out_v = out_d.rearrange("(p i) x -> p i x", i=NP)

    v = nc.vector
    gp = nc.gpsimd
    act = nc.scalar
    pe = nc.tensor

    with TileContext(nc) as tc:
        with (
            tc.tile_pool(name="main", bufs=1) as pool,
            # one SHARED psum pool of half-plane chunks ([128,512] fp32 =
            # 1 bank), 8-deep rotation across dual+primal: maximum lookahead
            # on the same 8 banks; unlocks a more Pool-shifted column split
            tc.tile_pool(name="psp", bufs=8, space="PSUM") as psp,
        ):
            o2t = pool.tile([P, NP, W], F16)
            ng0t = pool.tile([P, NP, W], F16)
            ng1t = pool.tile([P, NP, W], F16)
            qht = pool.tile([P, NP, W], F16)
            eyet = pool.tile([P, P], F16)
            neyet = pool.tile([P, P], F16)
            # sut: planes 0..7 = t data (col W = -1 guard for x+1 reads),
            # plane 8 = boundary row t[8p+8, x] (partition 127 stays -1)
            sut = pool.tile([P, NP + 1, WG], F16)
            # d1/d2: diff scratch, overwritten in place by t1/t2
            d1t = pool.tile([P, NP, W], F16)
            d2t = pool.tile([P, NP, W], F16)
            # na: planes 1..8 = -a data, plane 0 = boundary row -a[8p-1, x]
            nat = pool.tile([P, NP + 1, W], F16)
            # nb: cols 1..W = -b data, col 0 = zero guard for x-1 reads
            nbt = pool.tile([P, NP, WG], F16)
            dbt = pool.tile([P, NP, W], F16)
            outst = pool.tile([P, 4, W], F16)   # last-iter drain staging

            ENG = [(v, 0, XS), (gp, XS, W)]
            BLOCKS = [(b * NB, (b + 1) * NB) for b in range(NP // NB)]

            def u_(lo, hi, c0, c1):
                return sut[:, lo:hi, c0:c1]

            def unr(lo, hi, c0, c1):   # t[y+1, x] (plane 8 = boundary)
                return sut[:, lo + 1 : hi + 1, c0:c1]

            def unc(lo, hi, c0, c1):   # t[y, x+1] (col W = -1 guard)
                return sut[:, lo:hi, c0 + 1 : c1 + 1]

            def mk(tile):
                def f(lo, hi, c0, c1):
                    return tile[:, lo:hi, c0:c1]
                return f

            o2_, ng0_, ng1_, qh_ = mk(o2t), mk(ng0t), mk(ng1t), mk(qht)
            d1_, d2_, db_ = mk(d1t), mk(d2t), mk(dbt)

            def na_(lo, hi, c0, c1):     # -a data (planes 1..8)
                return nat[:, lo + 1 : hi + 1, c0:c1]

            def nb_(lo, hi, c0, c1):     # -b data (cols 1..W)
                return nbt[:, lo:hi, c0 + 1 : c1 + 1]

            def nbpc(lo, hi, c0, c1):    # -b[y, x-1] (col 0 = zero guard)
                return nbt[:, lo:hi, c0:c1]

            def emit(op_name, lo, hi, out_f, a_f, b_f):
                for eng, c0, c1 in ENG:
                    getattr(eng, op_name)(
                        out_f(lo, hi, c0, c1), a_f(lo, hi, c0, c1),
                        b_f(lo, hi, c0, c1))

            # guard-only memsets (interiors are fully overwritten)
            v.memset(sut[:, :, W:WG], -1.0)
            v.memset(sut[:, NP, :], -1.0)   # partition 127 keeps -1; the
            v.memset(nat[:, 0, :], 0.0)     # boundary DMAs rewrite the rest
            v.memset(nbt[:, :, 0:1], 0.0)
            nc.sync.dma_start(out=eyet[:, :], in_=eye_d)
            nc.sync.dma_start(out=neyet[:, :], in_=neye_d)
            for qq in range(4):
                nc.sync.dma_start(out=o2t[:, 2 * qq : 2 * qq + 2, :],
                                  in_=o2_v[:, 2 * qq : 2 * qq + 2, :])
            nc.sync.dma_start(out=ng1t[:, :, :], in_=g1_v)
            nc.sync.dma_start(out=ng0t[:, :, :], in_=g0_v)

            def dma_ushift():
                # su[p, 8, x] = t[8p+8, x] = su[p+1, 0, x]; row 127 stays -1
                nc.sync.dma_start(
                    out=sut[0 : P - 1, NP, 0:W], in_=sut[1:P, 0, 0:W]
                )

            def dma_ashift():
                # na[p, 0, x] = -a[8p-1] = na[p-1, 8, x]; row 0 stays 0
                nc.sync.dma_start(
                    out=nat[1:P, 0, 0:W], in_=nat[0 : P - 1, NP, 0:W]
                )

            for _rep in range(reps):
                if reps > 1:
                    v.memset(sut[:, 0:NP, 0:W], -1.0)
                for qq in range(4):
                    act.activation(
                        sut[:, 2 * qq : 2 * qq + 2, 0:W],
                        o2t[:, 2 * qq : 2 * qq + 2, :], AF.Tanh, scale=0.5,
                    )
                dma_ushift()

                for it in range(MAXITER):
                    last = it == MAXITER - 1
                    # --- dual TT: d = shifted diffs, t12 = -g * d ---
                    for lo, hi in BLOCKS:
                        emit("tensor_sub", lo, hi, d1_, unr, u_)
                        emit("tensor_mul", lo, hi, d1_, d1_, ng1_)
                        emit("tensor_sub", lo, hi, d2_, unc, u_)
                        emit("tensor_mul", lo, hi, d2_, d2_, ng0_)
                    # --- dual PE accumulate + relu drain (half-plane) ---
                    for i in range(NP):
                      for hc0, hc1 in [(0, 512), (512, W)]:
                        ps1 = psp.tile([P, 512], F32, name="ps")
                        if it > 0:
                            pe.matmul(ps1[:, :], eyet[:, :],
                                      qht[:, i, hc0:hc1], start=True,
                                      stop=False)
                        pe.matmul(ps1[:, :], eyet[:, :], d1t[:, i, hc0:hc1],
                                  start=(it == 0), stop=False)
                        pe.matmul(ps1[:, :], eyet[:, :], d2t[:, i, hc0:hc1],
                                  start=False, stop=True)
                        act.activation(qht[:, i, hc0:hc1], ps1[:, :], AF.Relu)
                    # --- primal TT ---
                    # na plane-8 row first so the boundary DMA fires early
                    for eng, c0, c1 in ENG:
                        eng.tensor_mul(
                            nat[:, NP, c0:c1],
                            ng1t[:, NP - 1, c0:c1],
                            qht[:, NP - 1, c0:c1],
                        )
                    dma_ashift()
                    for lo, hi in BLOCKS:
                        hi_w = min(hi, NP - 1)
                        if hi_w > lo:
                            emit("tensor_mul", lo, hi_w, na_, ng1_, qh_)
                        emit("tensor_mul", lo, hi, nb_, ng0_, qh_)
                        # db plane 7 goes via PE double-pass instead (below)
                        hi_db = min(hi, NP - 1)
                        if hi_db > lo:
                            emit("tensor_sub", lo, hi_db, db_, nb_, nbpc)
                    # --- primal PE accumulate + tanh drain / output ---
                    for i in range(NP):
                      for hc0, hc1 in [(0, 512), (512, W)]:
                        ps2 = psp.tile([P, 512], F32, name="ps")
                        pe.matmul(ps2[:, :], eyet[:, :], o2t[:, i, hc0:hc1],
                                  start=True, stop=False)
                        pe.matmul(ps2[:, :], eyet[:, :],
                                  nat[:, i + 1, hc0:hc1], start=False,
                                  stop=False)
                        pe.matmul(ps2[:, :], neyet[:, :], nat[:, i, hc0:hc1],
                                  start=False, stop=False)
                        if i < NP - 1:
                            pe.matmul(ps2[:, :], eyet[:, :],
                                      dbt[:, i, hc0:hc1], start=False,
                                      stop=True)
                        else:
                            pe.matmul(ps2[:, :], eyet[:, :],
                                      nbt[:, i, 1 + hc0 : 1 + hc1],
                                      start=False, stop=False)
                            pe.matmul(ps2[:, :], neyet[:, :],
                                      nbt[:, i, hc0:hc1], start=False,
                                      stop=True)
                        if last and reps == 1:
                            st = outst[:, i % 4, hc0:hc1]
                            act.activation(st, ps2[:, :], AF.Copy)
                            nc.sync.dma_start(out=out_v[:, i, hc0:hc1],
                                              in_=st)
                        else:
                            act.activation(
                                sut[:, i, hc0:hc1], ps2[:, :], AF.Tanh,
                                scale=0.5,
                            )
                            if i == 0 and hc1 == W:
                                dma_ushift()

    nc.compile()
    return nc


def kernel(o, vector_field, nabla_w, div_w):
    global LAST_RESULTS
    if "nc" not in _CACHE:
        _CACHE["nc"] = _build()
    nc = _CACHE["nc"]

    o2 = np.ascontiguousarray(
        (2.0 * np.asarray(o, dtype=np.float32)[:, 0]).astype(np.float16)
    )
    vf = np.asarray(vector_field, dtype=np.float32)
    s = np.float32(-1.0 / np.sqrt(2.0))
    ng0 = np.ascontiguousarray((vf[:, :, 0] * s).astype(np.float16))
    ng1 = np.ascontiguousarray((vf[:, :, 1] * s).astype(np.float16))
    eye = np.eye(P, dtype=np.float16)
    neye = -eye

    in_maps = [
        {"o2": o2[b], "ng0": ng0, "ng1": ng1, "eye": eye, "neye": neye}
        for b in range(B)
    ]
    res = bass_utils.run_bass_kernel_spmd(nc, in_maps, core_ids=list(range(B)))
    LAST_RESULTS = res
    return np.stack([r["out"] for r in res.results]).astype(np.float32)
